# revision 1
# baseline (speedup 1.0000x reference)
"""MultiHeadSelfAttentionWithRelativeBias on 8 TRN2 NeuronCores.

Sharding: data-parallel over batch (16 batches -> 2 per core).
Per-core pipeline (per batch, fully unrolled Tile program):
  - weights resident in SBUF (bf16); x^T loaded per batch (bf16).
  - V projection for all heads (lhsT = x^T chunk, rhs = Wv), packed into
    per-s-chunk "V_pad" tiles with a ones column per head: the ones column
    makes the attention*V matmul also emit softmax row-sums in row 64.
  - per head pair: Q^T/K^T chunks (lhsT = W chunk, rhs = x^T), Q scaled by
    1/sqrt(D) on ScalarE during the PSUM->SBUF copy.
  - per head, per k-chunk: scoresT[k,q] accumulates two passes in PSUM:
      (1) K^T.T @ Q^T (bf16, rows h2*64) and
      (2) relb_sw^T.T @ onehot (f32r, rows (1-h2)*64 — the bias features are
          stored half-swapped so passes alternate PE row groups and every
          LDWEIGHTS hides under the other pass's matmul).
    exp on ScalarE -> bf16, then the AV matmul accumulates immediately so
    exp tiles recycle fast.
  - normalize: reciprocal_approx_fast of row 64, f32r broadcast matmul
    (ones column x recip row), multiply on DVE -> bf16 out pair tiles.
  - O = out_all @ Wo (bf16) -> fp32 out.
"""
import numpy as np
import ml_dtypes


def et_prev_slice(et, nsl):
    return et[:, nsl]

import concourse.bass as bass
import concourse.mybir as mybir
import concourse.tile as tile
from concourse.bass_utils import run_bass_kernel_spmd
from concourse.vector_clock import VectorClock, ScopedClock

# ---------------------------------------------------------------- constants
B, S, E, H, D = 16, 1024, 1024, 16, 64
BOARD = 32
N_CORES = 8
BPC = B // N_CORES  # batches per core
PAIRS = H // 2      # head pairs (128 partition rows per pair)
KC = E // 128       # contraction chunks
F32 = mybir.dt.float32
F32R = mybir.dt.float32r
BF16 = mybir.dt.bfloat16
AF = mybir.ActivationFunctionType

# ------------------------------------------------- walrus compat workarounds


def _patched_drain_and_barrier(self, tick_clock, wait_clock):
    gc = tick_clock.global_clock
    n = len(gc)
    for p in range(n):
        if gc[p] <= 0:
            continue
        sub = VectorClock([0] * n)
        sub.require_at_least(p, gc[p])
        d = self.nc.sync.drain()
        wait_clock.add_sem_waits(d.ins, ScopedClock({None: sub}))
    self.nc.all_engine_barrier()
    popped = self.nc._tile_sem_poison_stack.pop()
    assert popped is self._sem_poison
    self.nc.clear_and_free_semaphores(list(self.sems.allocated().values()))
    self.nc.all_engine_barrier()


tile.TileContext._drain_and_barrier = _patched_drain_and_barrier


def _split_sync_waits(nc, max_waits=1):
    """This container's walrus accepts only one sync-wait per instruction;
    move excess waits onto preceding same-engine NOPs."""
    n_split = 0
    for bb in nc.m.functions[0].blocks:
        insts = bb.instructions
        i = 0
        while i < len(insts):
            inst = insts[i]
            si = inst.sync_info
            if si is not None and si.on_wait and len(si.on_wait) > max_waits:
                waits = list(si.on_wait)
                extra, keep = waits[:-max_waits], waits[-max_waits:]
                nops = []
                for j in range(0, len(extra), max_waits):
                    nops.append(mybir.InstNoOp(
                        name=f"I-{nc.next_id()}",
                        engine=inst.engine,
                        sync_info=mybir.SyncInfo(
                            on_wait=extra[j:j + max_waits], on_update=[]),
                        bass_nofuse=True,
                    ))
                si.on_wait = keep
                inst.sync_info = si
                insts[i:i] = nops
                i += len(nops)
                n_split += 1
            i += 1
    return n_split


# ------------------------------------------------------------- build kernel


def _build_nc():
    nc = bass.Bass("TRN2", target_bir_lowering=False, debug=False,
                   num_devices=1)

    xT = nc.dram_tensor("xT", [BPC, E, S], BF16, kind="ExternalInput")
    wq = nc.dram_tensor("Wq", [E, E], BF16, kind="ExternalInput")
    wk = nc.dram_tensor("Wk", [E, E], BF16, kind="ExternalInput")
    wv = nc.dram_tensor("Wv", [E, E], BF16, kind="ExternalInput")
    wo = nc.dram_tensor("Wo", [E, E], BF16, kind="ExternalInput")
    relb = nc.dram_tensor("relb_sw", [H * 64, S], F32R, kind="ExternalInput")
    onehot = nc.dram_tensor("onehotT", [64, S], F32R, kind="ExternalInput")
    ones64 = nc.dram_tensor("ones64", [1, 64], F32R, kind="ExternalInput")
    out = nc.dram_tensor("O", [BPC, S, E], F32, kind="ExternalOutput")

    with tile.TileContext(nc) as tc:
        with (
            tc.tile_pool(name="w", bufs=32) as wp,
            tc.tile_pool(name="xt", bufs=8) as xp,
            tc.tile_pool(name="oh", bufs=1) as ohp,
            tc.tile_pool(name="qk", bufs=8) as qkp,
            tc.tile_pool(name="exp", bufs=6) as ep,
            tc.tile_pool(name="vpad", bufs=8) as vp,
            tc.tile_pool(name="outp", bufs=8) as outp_pool,
            tc.tile_pool(name="small", bufs=2) as sp,
            tc.tile_pool(name="osb", bufs=3) as osp,
            tc.tile_pool(name="sc_ps", bufs=2, space="PSUM") as scps,
            tc.tile_pool(name="av_ps", bufs=1, space="PSUM") as avps,
            tc.tile_pool(name="mm_ps", bufs=2, space="PSUM") as mps,
        ):
            # resident weights: [e_in-chunk 128, e_out 1024] tiles
            # batch-0 x^T first: it gates the first projection matmuls
            xts_pref = []
            for k in range(KC):
                t = xp.tile([128, S], BF16, tag="xt", name=f"xtp{k}", bufs=8)
                nc.gpsimd.dma_start(t[:], xT.ap()[0, k * 128:(k + 1) * 128, :])
                xts_pref.append(t)
            wt = {}
            for wname, w in (("v", wv), ("q", wq), ("k", wk), ("o", wo)):
                for k in range(KC):
                    t = wp.tile([128, E], BF16, tag="w", name=f"w{wname}{k}",
                                bufs=32)
                    nc.gpsimd.dma_start(t[:], w.ap()[k * 128:(k + 1) * 128, :])
                    wt[wname, k] = t

            ones_sb = ohp.tile([1, 64], F32R, tag="ones", name="ones_sb")
            nc.gpsimd.dma_start(ones_sb[:], ones64.ap()[:, :])

            for b in range(BPC):
                # -------- x^T tiles for this batch
                if b == 0:
                    xts = xts_pref
                else:
                    xts = []
                    for k in range(KC):
                        t = xp.tile([128, S], BF16, tag="xt",
                                    name=f"xt{b}_{k}", bufs=8)
                        nc.gpsimd.dma_start(
                            t[:], xT.ap()[b, k * 128:(k + 1) * 128, :])
                        xts.append(t)

                # -------- V projection for all heads: out [s-chunk, e=1024]
                vpads = []
                for sc in range(KC):
                    vt = vp.tile([128, H * 65], BF16, tag="vpad",
                                 name=f"vpad{b}_{sc}", bufs=8)
                    for n in range(2):
                        nsl = slice(n * 512, (n + 1) * 512)
                        pv = mps.tile([128, 512], F32, tag="mm_ps",
                                      name=f"vps{b}_{sc}_{n}", bufs=2)
                        for k in range(KC):
                            nc.tensor.matmul(
                                pv[:], xts[k][:, sc * 128:(sc + 1) * 128],
                                wt["v", k][:, nsl], start=(k == 0),
                                stop=(k == KC - 1))
                        # 8 heads per half: interleave 64 V cols + ones col
                        dst = vt[:, n * 8 * 65:(n + 1) * 8 * 65].rearrange(
                            "p (h d) -> p h d", h=8)[:, :, 0:64]
                        src = pv[:].rearrange("p (h d) -> p h d", h=8)
                        nc.vector.tensor_copy(dst, src)
                        ones_dst = vt[:, n * 8 * 65:(n + 1) * 8 * 65].rearrange(
                            "p (h d) -> p h d", h=8)[:, :, 64:65]
                        nc.vector.memset(ones_dst, 1.0)
                    vpads.append(vt)

                outps = []
                pending = []
                for m in range(PAIRS):
                    # -------- per-head augmented Q/K tiles (f32r):
                    # rows 0:64 = head's Q^T or K^T, rows 64:128 = onehot
                    # (Q side) / relative-bias features (K side)
                    qa = [qkp.tile([128, S], F32R, tag="qk",
                                   name=f"qa{b}_{m}_{i}", bufs=8)
                          for i in range(2)]
                    ka = [qkp.tile([128, S], F32R, tag="qk",
                                   name=f"ka{b}_{m}_{i}", bufs=8)
                          for i in range(2)]
                    for i in range(2):
                        h = 2 * m + i
                        nc.gpsimd.dma_start(qa[i][64:128, :],
                                            onehot.ap()[:, :])
                        nc.gpsimd.dma_start(
                            ka[i][64:128, :],
                            relb.ap()[h * 64:(h + 1) * 64, :])
                    for pname, dsts in (("q", qa), ("k", ka)):
                        for n in range(2):
                            nsl = slice(n * 512, (n + 1) * 512)
                            pp = mps.tile([128, 512], F32, tag="mm_ps",
                                          name=f"{pname}ps{b}_{m}_{n}",
                                          bufs=2)
                            for k in range(KC):
                                nc.tensor.matmul(
                                    pp[:],
                                    wt[pname, k][:, m * 128:(m + 1) * 128],
                                    xts[k][:, nsl], start=(k == 0),
                                    stop=(k == KC - 1))
                            for i in range(2):
                                nc.vector.tensor_copy(
                                    dsts[i][0:64, nsl],
                                    pp[i * 64:(i + 1) * 64, :])

                    op_t = outp_pool.tile([128, S], BF16, tag="outp",
                                          name=f"op{b}_{m}", bufs=8)
                    outps.append(op_t)

                    for h2 in range(2):
                        h = 2 * m + h2
                        avp = avps.tile([65, S], F32, tag="av_ps",
                                        name=f"av{b}_{m}_{h2}", bufs=1)
                        prev_et = None
                        for kc in range(KC):
                            ksl = slice(kc * 128, (kc + 1) * 128)
                            sps = scps.tile([128, S], F32, tag="sc_ps",
                                            name=f"sps{b}_{m}_{h2}_{kc}",
                                            bufs=2)
                            for n in range(2):
                                nsl = slice(n * 512, (n + 1) * 512)
                                nc.tensor.matmul(sps[:, nsl],
                                                 ka[h2][:, ksl],
                                                 qa[h2][:, nsl],
                                                 start=True, stop=True)
                            et = ep.tile([128, S], BF16, tag="exp",
                                         name=f"exp{b}_{m}_{h2}_{kc}", bufs=6)
                            nc.scalar.activation(et[:], sps[:], AF.Exp)
                            for n in range(2):
                                nsl = slice(n * 512, (n + 1) * 512)
                                nc.tensor.matmul(
                                    avp[:, nsl],
                                    vpads[kc][:, h * 65:(h + 1) * 65],
                                    et[:, nsl], start=(kc == 0),
                                    stop=(kc == KC - 1))
                            if kc == 2 and pending:
                                pending.pop(0)()
                        # free avp fast: copy rows + ln(sums); the rest of
                        # the normalization is deferred into the next head's
                        # score loop so the PE never stalls on the ACT chain
                        un = sp.tile([64, S], BF16, tag="un",
                                     name=f"un{b}_{m}_{h2}", bufs=3)
                        nc.vector.tensor_copy(un[:], avp[0:64, :])
                        lns = sp.tile([1, S], F32, tag="lns",
                                      name=f"lns{b}_{m}_{h2}", bufs=2)
                        nc.scalar.activation(lns[:], avp[64:65, :], AF.Ln)
                        rec_r = sp.tile([1, S], F32R, tag="recr",
                                        name=f"recr{b}_{m}_{h2}", bufs=3)
                        # 1/s = exp(-ln(s)); Ln+Exp share one ACT table set
                        nc.scalar.activation(rec_r[:], lns[:], AF.Exp,
                                             scale=-1.0)

                        def _normalize(un=un, rec_r=rec_r, op_t=op_t, b=b,
                                       m=m, h2=h2):
                            for n in range(2):
                                nsl = slice(n * 512, (n + 1) * 512)
                                bps_t = mps.tile([64, 512], F32, tag="mm_ps",
                                                 name=f"bcp{b}_{m}_{h2}_{n}",
                                                 bufs=2)
                                nc.tensor.matmul(bps_t[:], ones_sb[:],
                                                 rec_r[:, nsl], start=True,
                                                 stop=True)
                                nc.vector.tensor_mul(
                                    op_t[h2 * 64:(h2 + 1) * 64, nsl],
                                    un[:, nsl], bps_t[:])

                        pending.append(_normalize)

                for fn in pending:
                    fn()
                pending = []

                # -------- output projection: O = out_all @ Wo
                for ms in range(KC):
                    msl = slice(ms * 128, (ms + 1) * 128)
                    for n in range(2):
                        nsl = slice(n * 512, (n + 1) * 512)
                        po = mps.tile([128, 512], F32, tag="mm_ps",
                                      name=f"ops{b}_{ms}_{n}", bufs=2)
                        for p in range(PAIRS):
                            nc.tensor.matmul(
                                po[:], outps[p][:, msl], wt["o", p][:, nsl],
                                start=(p == 0), stop=(p == PAIRS - 1))
                        ot = osp.tile([128, 512], F32, tag="osb",
                                      name=f"ot{b}_{ms}_{n}", bufs=3)
                        nc.scalar.copy(ot[:], po[:])
                        nc.gpsimd.dma_start(out.ap()[b, msl, nsl], ot[:])

    _split_sync_waits(nc)
    return nc


_NC = None


def _get_nc():
    global _NC
    if _NC is None:
        _NC = _build_nc()
    return _NC


# ----------------------------------------------------------- host-side prep


def _host_prep(x, Wq, Wk, Wv, Wo, rel_bias):
    bf = ml_dtypes.bfloat16
    # relative-bias features: for head h, row a (a<32): rel_bias[h, j//32-a+31]
    # row 32+c: rel_bias[h, j%32-c+31]  (j = key index). Stored half-swapped
    # per pair: pair m rows 0:64 = head 2m+1, rows 64:128 = head 2m.
    j = np.arange(S)
    jr, jc = j // BOARD, j % BOARD
    a = np.arange(BOARD)
    relb = np.empty((H, 64, S), dtype=np.float32)
    for h in range(H):
        relb[h, 0:32, :] = rel_bias[h][jr[None, :] - a[:, None] + BOARD - 1]
        relb[h, 32:64, :] = rel_bias[h][jc[None, :] - a[:, None] + BOARD - 1]
    relb_sw = relb.reshape(H * 64, S)  # bisect: unswapped

    onehot = np.zeros((64, S), dtype=np.float32)
    onehot[jr, j] = 1.0          # rows 0:32 one-hot of q//32
    onehot[32 + jc, j] = 1.0     # rows 32:64 one-hot of q%32

    ones64 = np.ones((1, 64), dtype=np.float32)

    wq_b = np.ascontiguousarray((Wq * 0.125).astype(bf))  # fold 1/sqrt(D)
    wk_b = np.ascontiguousarray(Wk.astype(bf))
    wv_b = np.ascontiguousarray(Wv.astype(bf))
    wo_b = np.ascontiguousarray(Wo.astype(bf))

    in_maps = []
    for c in range(N_CORES):
        xc = x[c * BPC:(c + 1) * BPC]                    # [BPC, S, E]
        xt = np.ascontiguousarray(xc.transpose(0, 2, 1).astype(bf))
        in_maps.append({
            "xT": xt, "Wq": wq_b, "Wk": wk_b, "Wv": wv_b, "Wo": wo_b,
            "relb_sw": relb_sw, "onehotT": onehot, "ones64": ones64,
        })
    return in_maps


def kernel(x, Wq, Wk, Wv, Wo, rel_bias, _trace=False):
    nc = _get_nc()
    in_maps = _host_prep(np.asarray(x), np.asarray(Wq), np.asarray(Wk),
                         np.asarray(Wv), np.asarray(Wo), np.asarray(rel_bias))
    res = run_bass_kernel_spmd(nc, in_maps, core_ids=list(range(N_CORES)),
                               trace=_trace)
    out = np.concatenate([res.results[c]["O"] for c in range(N_CORES)], axis=0)
    if _trace:
        kernel.last_exec_time_ns = res.exec_time_ns
        kernel.last_results = res
    return out

